# revision 14
# baseline (speedup 1.0000x reference)
"""CTREmbedding Trainium2 kernel.

out[b,l,m,e] = interval-embedding interpolation:
    v  = (l < traj_length[b])                       in {0,1}
    ds = v ? mat2[traj_location[b,l]-1, m] : 0      (ds' = ds/1000)
    dt = vector[b,l]                                (dt' = dt/86400)
    out = ds'*S1[e] + C0[e] + v*Cv[e] + dt'*Ct[e] + v*dt'*Cvt[e]

Per (b,l) pair the [M,E] block is computed by a K=58 fp16 matmul in
double-fp16 (hi+lo split) precision:
    each operand x is split x = h(x) + l(x) with h/l fp16; the product
    x*y is carried as h(x)h(y) + h(x)l(y) + l(x)h(y) on separate K rows
    (the dropped l*l term is O(2^-22)).  K rows:
      0:16   dsT_hi x blockdiag(S1_hi)    (dsT[j,p] = ds'[16p+j])
      16:32  dsT_hi x blockdiag(S1_lo)
      32:48  dsT_lo x blockdiag(S1_hi)
      48:58  per-pair scalars (1,1,v,v,dth,dth,dtl,v*dth,v*dth,v*dtl)
             x C-basis rows (C0h,C0l,Cvh,Cvl,Cth,Ctl,Cth,Cvth,Cvtl,Cvth)
    out [128,800]: out[p, j*50+e] = value at m=16p+j  -> partition p owns
    m in [16p,16p+16) contiguous output bytes.

Why double-fp16: the output contains near-zero cancellations between O(1)
terms, so the correctness metric is sensitive to ABSOLUTE compute error;
bf16 operands (4e-3 abs) fail it, while fp32 operands stream 4x slower
through the PE (76us vs 17us per core).  K=58 <= 128 keeps the matmul a
single pass, so the hi/lo expansion costs no PE time.  The fp16 OUTPUT is
safe (per-element relative rounding) and halves the HBM write traffic vs
f32; the host upcasts the returned output to f32.

The dsT blocks are gathered on HOST (only 400 of 4096 mat2 rows are used;
3.2MB of host work) and shipped in the consts input — a device-side SWDGE
indirect-gather chain measured 70us serialized and did not overlap with
the output DMAs, doubling kernel time.

Sharding: 400 (b,l) pairs, 50 per core on 8 cores; each core writes a
contiguous [50, M*E] slice.  The kernel is output-DMA bound (~10.2MB/core
at ~358GB/s); group sizes keep the output DMA saturated from ~7us on.
"""

import numpy as np

B, L, M, E, NLOC = 4, 100, 2048, 50, 4096
EX_SU, EX_SL, EX_TU, EX_TL = 1000.0, 0.0, 86400.0, 0.0

N_CORES = 8
PAIRS = B * L                      # 400
PPC = PAIRS // N_CORES             # 50 pairs per core
JJ = 16                            # m-values per partition
PCH = M // JJ                      # 128 partitions
FREE = JJ * E                      # 800 floats per partition per pair
K = 58                             # matmul contraction rows
# group sizes: DMA consumes a pair in ~569ns, the copy engines produce one
# in ~558ns, so after a small leading group the output DMA never starves;
# small tail groups shorten the drain.
SIZES = [1, 1, 2, 2] + [3] * 14 + [1, 1]    # sum = 50
LW = PPC * PCH                     # 6400: dsT+sigma region width
CW = LW + FREE                     # 7200: + rhs table columns

_cache = {}


def _build_bass(out_dt_name="float16", in_dt_name="float16", sizes=None,
                split=(384, 416), ap="inter", obufs=12, pbufs=4,
                dmaq="sync", nmm=2):
    import concourse.tile as tile
    from concourse import bacc, mybir
    from concourse.tile import add_dep_helper

    f32 = mybir.dt.float32
    in_dt = getattr(mybir.dt, in_dt_name)
    out_dt = getattr(mybir.dt, out_dt_name)
    sizes = sizes or SIZES
    assert sum(sizes) == PPC

    nc = bacc.Bacc("TRN2", target_bir_lowering=False, debug=False,
                   num_devices=N_CORES)
    # consts[:, 0:800] = rhs table; consts[0:48, 800:] = dsT hi/hi/lo;
    # consts[48:58, 800:] = sigma.  rhs-first layout lets one leading DMA
    # cover the rhs table plus the first pairs' columns contiguously.
    consts = nc.declare_dram_parameter("consts", [K, CW], in_dt,
                                       isOutput=False)
    # ap="inter": pair-major [PPC, M*E] with interleaved (p,q,r) DMA APs —
    # measured faster on HW than the partition-major "plain" layout.
    if ap == "inter":
        out = nc.declare_dram_parameter("out", [PPC, M * E], out_dt,
                                        isOutput=True)
    else:
        out = nc.declare_dram_parameter("out", [PCH, PPC * FREE], out_dt,
                                        isOutput=True)

    with tile.TileContext(nc) as tc:
        with (
            tc.tile_pool(name="const", bufs=1) as cpool,
            tc.tile_pool(name="outp", bufs=obufs) as opool,
            tc.tile_pool(name="psum", bufs=pbufs, space="PSUM") as ppool,
        ):
            # ACT warmup at t~0: a dependency-free memset + tiny scalar.copy
            # pulls the 1.3us LoadActFuncSet under the input DMA instead of
            # serializing it before the first real PSUM->SBUF copy.
            wact = cpool.tile([1, 16], in_dt)
            nc.vector.memset(wact[0:1, 0:8], 0)
            nc.scalar.copy(out=wact[0:1, 8:16], in_=wact[0:1, 0:8])

            lhs_sb = cpool.tile([K, CW], in_dt)
            # three input chunks: a tiny leading DMA (rhs table + 2 pairs)
            # unblocks the first matmul ASAP; the next 6 pairs land via the
            # other HWDGE ring so descriptor generation overlaps; the bulk
            # follows on the first ring.
            DS0 = 2 * PCH
            DS1 = 8 * PCH
            dma_ds0 = nc.sync.dma_start(out=lhs_sb[:, 0:FREE + DS0],
                                        in_=consts[:, 0:FREE + DS0])
            nc.scalar.dma_start(out=lhs_sb[:, FREE + DS0:FREE + DS1],
                                in_=consts[:, FREE + DS0:FREE + DS1])
            nc.sync.dma_start(out=lhs_sb[:, FREE + DS1:CW],
                              in_=consts[:, FREE + DS1:CW])
            # warmup matmul absorbs the leading input-DMA waits on PE and
            # starts the HAM ramp before the first real pair arrives
            wps = ppool.tile([PCH, FREE], f32, tag="ps")
            wmm = nc.tensor.matmul(
                out=wps[0:4, 0:4], lhsT=lhs_sb[0:K, 0:4],
                rhs=lhs_sb[0:K, 4:8], start=True, stop=True,
            )
            add_dep_helper(wmm.ins, dma_ds0.ins, True, "absorb dsT wait")

            _loop_body(nc, opool, ppool, lhs_sb, sizes, split,
                       out_dt, out, ap, dmaq, nmm)
    nc.compile()
    return nc


def _loop_body(nc, opool, ppool, lhs_sb, sizes, split, out_dt, out, ap,
               dmaq="sync", nmm=2):
    from concourse import mybir

    f32 = mybir.dt.float32
    s0, s1 = split
    i0 = 0
    for g, ng in enumerate(sizes):
        out_sb = opool.tile([PCH, ng * FREE], out_dt, tag="out_sb")
        for q in range(ng):
            i = i0 + q
            lhsT = lhs_sb[0:K, FREE + i * PCH: FREE + (i + 1) * PCH]
            dst = out_sb[:, q * FREE: (q + 1) * FREE]
            ps = ppool.tile([PCH, FREE], f32, tag="ps")
            if nmm == 1:
                nc.tensor.matmul(
                    out=ps[:, :], lhsT=lhsT,
                    rhs=lhs_sb[0:K, 0:FREE],
                    start=True, stop=True,
                )
            else:
                nc.tensor.matmul(
                    out=ps[:, 0:512], lhsT=lhsT,
                    rhs=lhs_sb[0:K, 0:512],
                    start=True, stop=True,
                )
                nc.tensor.matmul(
                    out=ps[:, 512:FREE], lhsT=lhsT,
                    rhs=lhs_sb[0:K, 512:FREE],
                    start=True, stop=True,
                )
            # ACT takes the leading block (covered by the first matmul, so
            # the laggier ACT sem-path starts earliest); DVE takes the rest.
            if s0 > 0:
                nc.scalar.copy(out=dst[:, 0:s0], in_=ps[:, 0:s0])
            if s0 < FREE:
                nc.vector.tensor_copy(out=dst[:, s0:FREE], in_=ps[:, s0:FREE])
        eng = nc.sync if (dmaq == "sync" or g % 2 == 0) else nc.scalar
        if ap == "inter":
            dram_ap = out[i0: i0 + ng, :].rearrange("q (p r) -> p q r",
                                                    p=PCH)
            sb_ap = out_sb[:, 0: ng * FREE].rearrange("p (q r) -> p q r",
                                                      q=ng)
            eng.dma_start(out=dram_ap, in_=sb_ap)
        else:
            eng.dma_start(out=out[:, i0 * FREE: (i0 + ng) * FREE],
                          in_=out_sb[:, 0: ng * FREE])
        i0 += ng


def _split16(x):
    """Double-fp16 split: x ~= hi + lo with hi/lo fp16."""
    x = np.asarray(x, dtype=np.float32)
    hi = x.astype(np.float16)
    lo = (x - hi.astype(np.float32)).astype(np.float16)
    return hi, lo


def _host_prep(inputs):
    traj_location = np.asarray(inputs["traj_location"]).astype(np.int64)
    mat2 = np.asarray(inputs["mat2"], dtype=np.float32)
    vector = np.asarray(inputs["vector"], dtype=np.float32)
    traj_length = np.asarray(inputs["traj_length"]).astype(np.int64)
    emb_su = np.asarray(inputs["emb_su"], dtype=np.float32)
    emb_sl = np.asarray(inputs["emb_sl"], dtype=np.float32)
    emb_tu = np.asarray(inputs["emb_tu"], dtype=np.float32)
    emb_tl = np.asarray(inputs["emb_tl"], dtype=np.float32)

    # ---- host prep: O(B*L) scalars + a 3.2MB row gather ----
    valid = (np.arange(L)[None, :] < traj_length[:, None]).reshape(-1)  # [400]
    v = valid.astype(np.float32)
    dtn = vector.reshape(-1) / np.float32(EX_TU - EX_TL)   # dt' in [0,1)
    loc0 = (traj_location.reshape(-1) - 1).astype(np.int64)
    dsn = np.where(valid[:, None], mat2[loc0], np.float32(0.0)) \
        / np.float32(EX_SU - EX_SL)                        # ds' in [0,1)

    # normalized basis vectors (all O(1))
    S1 = emb_su[1] - emb_sl[1]
    C0 = emb_sl[0] + emb_tl[0]
    Cv = (emb_sl[1] + emb_tl[1]) - C0
    Ct = emb_tu[0] - emb_tl[0]
    Cvt = (emb_tu[1] - emb_tl[1]) - Ct

    S1h, S1l = _split16(S1)
    C0h, C0l = _split16(C0)
    Cvh, Cvl = _split16(Cv)
    Cth, Ctl = _split16(Ct)
    Cvth, Cvtl = _split16(Cvt)
    dsh, dsl = _split16(dsn)        # [400, M]
    dth, dtl = _split16(dtn)        # [400]

    # rhs table [58, 800]
    rhstab = np.zeros((K, FREE), np.float16)
    for j in range(JJ):
        rhstab[j, j * E: (j + 1) * E] = S1h
        rhstab[16 + j, j * E: (j + 1) * E] = S1l
        rhstab[32 + j, j * E: (j + 1) * E] = S1h
    for r, vec in enumerate([C0h, C0l, Cvh, Cvl, Cth, Ctl, Cth,
                             Cvth, Cvtl, Cvth]):
        rhstab[48 + r, :] = np.tile(vec, JJ)

    f32v = v
    dthf = dth.astype(np.float32)
    dtlf = dtl.astype(np.float32)
    ones = np.ones(PAIRS, np.float32)
    # sigma rows match rhs rows 48..57 (v in {0,1} keeps products exact)
    sig_all = np.stack([ones, ones, f32v, f32v, dthf, dthf, dtlf,
                        f32v * dthf, f32v * dthf, f32v * dtlf])  # [10, 400]

    in_maps = []
    for c in range(N_CORES):
        sl = slice(c * PPC, (c + 1) * PPC)
        # dsT[j, i*128 + p] = ds'[i, 16p + j]
        dshT = np.ascontiguousarray(
            dsh[sl].reshape(PPC, PCH, JJ).transpose(2, 0, 1).reshape(JJ, LW)
        )
        dslT = np.ascontiguousarray(
            dsl[sl].reshape(PPC, PCH, JJ).transpose(2, 0, 1).reshape(JJ, LW)
        )
        sigma = np.repeat(sig_all[:, sl], PCH, axis=1)
        consts = np.zeros((K, CW), np.float16)
        consts[:, 0:FREE] = rhstab
        consts[0:16, FREE:CW] = dshT
        consts[16:32, FREE:CW] = dshT
        consts[32:48, FREE:CW] = dslT
        consts[48:58, FREE:CW] = sigma.astype(np.float16)
        in_maps.append({"consts": consts})
    return in_maps


def kernel(**inputs):
    from concourse.bass_utils import run_bass_kernel_spmd

    in_maps = _host_prep(inputs)
    if "nc" not in _cache:
        _cache["nc"] = _build_bass()
    res = run_bass_kernel_spmd(_cache["nc"], in_maps,
                               core_ids=list(range(N_CORES)))
    parts = [np.asarray(res.results[c]["out"]).astype(np.float32)
             .reshape(PPC, M, E) for c in range(N_CORES)]
    return np.concatenate(parts, axis=0).reshape(B, L, M, E)


# revision 15
# speedup vs baseline: 1.6366x; 1.6366x over previous
"""CTREmbedding Trainium2 kernel.

out[b,l,m,e] = interval-embedding interpolation:
    v  = (l < traj_length[b])                       in {0,1}
    ds = v ? mat2[traj_location[b,l]-1, m] : 0      (ds' = ds/1000)
    dt = vector[b,l]                                (dt' = dt/86400)
    out = ds'*S1[e] + C0[e] + v*Cv[e] + dt'*Ct[e] + v*dt'*Cvt[e]

Per (b,l) pair the [M,E] block is computed by a K=58 fp16 matmul in
double-fp16 (hi+lo split) precision:
    each operand x is split x = h(x) + l(x) with h/l fp16; the product
    x*y is carried as h(x)h(y) + h(x)l(y) + l(x)h(y) on separate K rows
    (the dropped l*l term is O(2^-22)).  K rows:
      0:16   dsT_hi x blockdiag(S1_hi)    (dsT[j,p] = ds'[16p+j])
      16:32  dsT_hi x blockdiag(S1_lo)
      32:48  dsT_lo x blockdiag(S1_hi)
      48:58  per-pair scalars (1,1,v,v,dth,dth,dtl,v*dth,v*dth,v*dtl)
             x C-basis rows (C0h,C0l,Cvh,Cvl,Cth,Ctl,Cth,Cvth,Cvtl,Cvth)
    out [128,800]: out[p, j*50+e] = value at m=16p+j  -> partition p owns
    m in [16p,16p+16) contiguous output bytes.

Why double-fp16: the output contains near-zero cancellations between O(1)
terms, so the correctness metric is sensitive to ABSOLUTE compute error;
bf16 operands (4e-3 abs) fail it, while fp32 operands stream 4x slower
through the PE (76us vs 17us per core).  K=58 <= 128 keeps the matmul a
single pass, so the hi/lo expansion costs no PE time.  The fp16 OUTPUT is
safe (per-element relative rounding) and halves the HBM write traffic vs
f32; the host upcasts the returned output to f32.

The dsT blocks are gathered on HOST (only 400 of 4096 mat2 rows are used;
3.2MB of host work) and shipped in the consts input — a device-side SWDGE
indirect-gather chain measured 70us serialized and did not overlap with
the output DMAs, doubling kernel time.

Sharding: 400 (b,l) pairs, 50 per core on 8 cores; each core writes a
contiguous [50, M*E] slice.  The kernel is output-DMA bound (~10.2MB/core
at ~358GB/s); group sizes keep the output DMA saturated from ~7us on.
"""

import numpy as np

B, L, M, E, NLOC = 4, 100, 2048, 50, 4096
EX_SU, EX_SL, EX_TU, EX_TL = 1000.0, 0.0, 86400.0, 0.0

N_CORES = 8
PAIRS = B * L                      # 400
PPC = PAIRS // N_CORES             # 50 pairs per core
JJ = 16                            # m-values per partition
PCH = M // JJ                      # 128 partitions
FREE = JJ * E                      # 800 floats per partition per pair
K = 58                             # matmul contraction rows
# group sizes: DMA consumes a pair in ~569ns, the copy engines produce one
# in ~558ns, so after a small leading group the output DMA never starves;
# small tail groups shorten the drain.
SIZES = [1, 1, 2, 2] + [3] * 14 + [1, 1]    # sum = 50
LW = PPC * PCH                     # 6400: dsT+sigma region width
CW = LW + FREE                     # 7200: + rhs table columns

_cache = {}


def _build_bass(out_dt_name="float16", in_dt_name="float16", sizes=None,
                split=(416, 384), ap="inter", obufs=12, pbufs=4,
                dmaq="sync", nmm=2):
    import concourse.tile as tile
    from concourse import bacc, mybir
    from concourse.tile import add_dep_helper

    f32 = mybir.dt.float32
    in_dt = getattr(mybir.dt, in_dt_name)
    out_dt = getattr(mybir.dt, out_dt_name)
    sizes = sizes or SIZES
    assert sum(sizes) == PPC

    nc = bacc.Bacc("TRN2", target_bir_lowering=False, debug=False,
                   num_devices=N_CORES)
    # consts[:, 0:800] = rhs table; consts[0:48, 800:] = dsT hi/hi/lo;
    # consts[48:58, 800:] = sigma.  rhs-first layout lets one leading DMA
    # cover the rhs table plus the first pairs' columns contiguously.
    consts = nc.declare_dram_parameter("consts", [K, CW], in_dt,
                                       isOutput=False)
    # ap="inter": pair-major [PPC, M*E] with interleaved (p,q,r) DMA APs —
    # measured faster on HW than the partition-major "plain" layout.
    if ap == "inter":
        out = nc.declare_dram_parameter("out", [PPC, M * E], out_dt,
                                        isOutput=True)
    else:
        out = nc.declare_dram_parameter("out", [PCH, PPC * FREE], out_dt,
                                        isOutput=True)

    with tile.TileContext(nc) as tc:
        with (
            tc.tile_pool(name="const", bufs=1) as cpool,
            tc.tile_pool(name="outp", bufs=obufs) as opool,
            tc.tile_pool(name="psum", bufs=pbufs, space="PSUM") as ppool,
        ):
            # ACT warmup at t~0: a dependency-free memset + tiny scalar.copy
            # pulls the 1.3us LoadActFuncSet under the input DMA instead of
            # serializing it before the first real PSUM->SBUF copy.
            wact = cpool.tile([1, 16], in_dt)
            nc.vector.memset(wact[0:1, 0:8], 0)
            nc.scalar.copy(out=wact[0:1, 8:16], in_=wact[0:1, 0:8])

            lhs_sb = cpool.tile([K, CW], in_dt)
            # three input chunks: a tiny leading DMA (rhs table + 2 pairs)
            # unblocks the first matmul ASAP; the next 6 pairs land via the
            # other HWDGE ring so descriptor generation overlaps; the bulk
            # follows on the first ring.
            DS0 = 2 * PCH
            DS1 = 8 * PCH
            dma_ds0 = nc.sync.dma_start(out=lhs_sb[:, 0:FREE + DS0],
                                        in_=consts[:, 0:FREE + DS0])
            nc.scalar.dma_start(out=lhs_sb[:, FREE + DS0:FREE + DS1],
                                in_=consts[:, FREE + DS0:FREE + DS1])
            nc.sync.dma_start(out=lhs_sb[:, FREE + DS1:CW],
                              in_=consts[:, FREE + DS1:CW])
            # warmup matmul absorbs the leading input-DMA waits on PE and
            # starts the HAM ramp before the first real pair arrives
            wps = ppool.tile([PCH, FREE], f32, tag="ps")
            wmm = nc.tensor.matmul(
                out=wps[0:4, 0:4], lhsT=lhs_sb[0:K, 0:4],
                rhs=lhs_sb[0:K, 4:8], start=True, stop=True,
            )
            add_dep_helper(wmm.ins, dma_ds0.ins, True, "absorb dsT wait")

            _loop_body(nc, opool, ppool, lhs_sb, sizes, split,
                       out_dt, out, ap, dmaq, nmm)
    nc.compile()
    return nc


def _loop_body(nc, opool, ppool, lhs_sb, sizes, split, out_dt, out, ap,
               dmaq="sync", nmm=2):
    from concourse import mybir

    f32 = mybir.dt.float32
    s0, s1 = split
    i0 = 0
    for g, ng in enumerate(sizes):
        out_sb = opool.tile([PCH, ng * FREE], out_dt, tag="out_sb")
        for q in range(ng):
            i = i0 + q
            lhsT = lhs_sb[0:K, FREE + i * PCH: FREE + (i + 1) * PCH]
            dst = out_sb[:, q * FREE: (q + 1) * FREE]
            ps = ppool.tile([PCH, FREE], f32, tag="ps")
            if nmm == 1:
                nc.tensor.matmul(
                    out=ps[:, :], lhsT=lhsT,
                    rhs=lhs_sb[0:K, 0:FREE],
                    start=True, stop=True,
                )
            else:
                nc.tensor.matmul(
                    out=ps[:, 0:512], lhsT=lhsT,
                    rhs=lhs_sb[0:K, 0:512],
                    start=True, stop=True,
                )
                nc.tensor.matmul(
                    out=ps[:, 512:FREE], lhsT=lhsT,
                    rhs=lhs_sb[0:K, 512:FREE],
                    start=True, stop=True,
                )
            # ACT takes the leading block (covered by the first matmul, so
            # the laggier ACT sem-path starts earliest); DVE takes the rest.
            if s0 > 0:
                nc.scalar.copy(out=dst[:, 0:s0], in_=ps[:, 0:s0])
            if s0 < FREE:
                nc.vector.tensor_copy(out=dst[:, s0:FREE], in_=ps[:, s0:FREE])
        eng = nc.sync if (dmaq == "sync" or g % 2 == 0) else nc.scalar
        if ap == "inter":
            dram_ap = out[i0: i0 + ng, :].rearrange("q (p r) -> p q r",
                                                    p=PCH)
            sb_ap = out_sb[:, 0: ng * FREE].rearrange("p (q r) -> p q r",
                                                      q=ng)
            eng.dma_start(out=dram_ap, in_=sb_ap)
        else:
            eng.dma_start(out=out[:, i0 * FREE: (i0 + ng) * FREE],
                          in_=out_sb[:, 0: ng * FREE])
        i0 += ng


def _split16(x):
    """Double-fp16 split: x ~= hi + lo with hi/lo fp16."""
    x = np.asarray(x, dtype=np.float32)
    hi = x.astype(np.float16)
    lo = (x - hi.astype(np.float32)).astype(np.float16)
    return hi, lo


def _host_prep(inputs):
    traj_location = np.asarray(inputs["traj_location"]).astype(np.int64)
    mat2 = np.asarray(inputs["mat2"], dtype=np.float32)
    vector = np.asarray(inputs["vector"], dtype=np.float32)
    traj_length = np.asarray(inputs["traj_length"]).astype(np.int64)
    emb_su = np.asarray(inputs["emb_su"], dtype=np.float32)
    emb_sl = np.asarray(inputs["emb_sl"], dtype=np.float32)
    emb_tu = np.asarray(inputs["emb_tu"], dtype=np.float32)
    emb_tl = np.asarray(inputs["emb_tl"], dtype=np.float32)

    # ---- host prep: O(B*L) scalars + a 3.2MB row gather ----
    valid = (np.arange(L)[None, :] < traj_length[:, None]).reshape(-1)  # [400]
    v = valid.astype(np.float32)
    dtn = vector.reshape(-1) / np.float32(EX_TU - EX_TL)   # dt' in [0,1)
    loc0 = (traj_location.reshape(-1) - 1).astype(np.int64)
    dsn = np.where(valid[:, None], mat2[loc0], np.float32(0.0)) \
        / np.float32(EX_SU - EX_SL)                        # ds' in [0,1)

    # normalized basis vectors (all O(1))
    S1 = emb_su[1] - emb_sl[1]
    C0 = emb_sl[0] + emb_tl[0]
    Cv = (emb_sl[1] + emb_tl[1]) - C0
    Ct = emb_tu[0] - emb_tl[0]
    Cvt = (emb_tu[1] - emb_tl[1]) - Ct

    S1h, S1l = _split16(S1)
    C0h, C0l = _split16(C0)
    Cvh, Cvl = _split16(Cv)
    Cth, Ctl = _split16(Ct)
    Cvth, Cvtl = _split16(Cvt)
    dsh, dsl = _split16(dsn)        # [400, M]
    dth, dtl = _split16(dtn)        # [400]

    # rhs table [58, 800]
    rhstab = np.zeros((K, FREE), np.float16)
    for j in range(JJ):
        rhstab[j, j * E: (j + 1) * E] = S1h
        rhstab[16 + j, j * E: (j + 1) * E] = S1l
        rhstab[32 + j, j * E: (j + 1) * E] = S1h
    for r, vec in enumerate([C0h, C0l, Cvh, Cvl, Cth, Ctl, Cth,
                             Cvth, Cvtl, Cvth]):
        rhstab[48 + r, :] = np.tile(vec, JJ)

    f32v = v
    dthf = dth.astype(np.float32)
    dtlf = dtl.astype(np.float32)
    ones = np.ones(PAIRS, np.float32)
    # sigma rows match rhs rows 48..57 (v in {0,1} keeps products exact)
    sig_all = np.stack([ones, ones, f32v, f32v, dthf, dthf, dtlf,
                        f32v * dthf, f32v * dthf, f32v * dtlf])  # [10, 400]

    in_maps = []
    for c in range(N_CORES):
        sl = slice(c * PPC, (c + 1) * PPC)
        # dsT[j, i*128 + p] = ds'[i, 16p + j]
        dshT = np.ascontiguousarray(
            dsh[sl].reshape(PPC, PCH, JJ).transpose(2, 0, 1).reshape(JJ, LW)
        )
        dslT = np.ascontiguousarray(
            dsl[sl].reshape(PPC, PCH, JJ).transpose(2, 0, 1).reshape(JJ, LW)
        )
        sigma = np.repeat(sig_all[:, sl], PCH, axis=1)
        consts = np.zeros((K, CW), np.float16)
        consts[:, 0:FREE] = rhstab
        consts[0:16, FREE:CW] = dshT
        consts[16:32, FREE:CW] = dshT
        consts[32:48, FREE:CW] = dslT
        consts[48:58, FREE:CW] = sigma.astype(np.float16)
        in_maps.append({"consts": consts})
    return in_maps


def kernel(**inputs):
    from concourse.bass_utils import run_bass_kernel_spmd

    in_maps = _host_prep(inputs)
    if "nc" not in _cache:
        _cache["nc"] = _build_bass()
    res = run_bass_kernel_spmd(_cache["nc"], in_maps,
                               core_ids=list(range(N_CORES)))
    parts = [np.asarray(res.results[c]["out"]).astype(np.float32)
             .reshape(PPC, M, E) for c in range(N_CORES)]
    return np.concatenate(parts, axis=0).reshape(B, L, M, E)
